# revision 1
# baseline (speedup 1.0000x reference)
"""AssocScan Trainium2 kernel: out[:, t] = gates[:, t] * out[:, t-1] + inputs[:, t].

Strategy: the recurrence is independent per (b, d) lane (B*D = 4096 lanes,
N = 4096 steps). The DVE `tensor_tensor_scan` instruction computes exactly
this recurrence along the free dimension, one lane per partition.

Sharding: lanes are split evenly across the 8 NeuronCores (512 lanes each).
During host-side sharding the (B, N, D) inputs are transposed to lane-major
(B*D, N) so every device DMA is fully contiguous (time series per lane
contiguous in DRAM), and cast to bf16 (the scan state and outputs stay
fp32). Each core streams 4 chunks of [128 lanes, 4096 steps]: chunk loads
on the sync-engine DMA ring (FIFO => completions in scan order), scans on
the VectorEngine chasing the loads, stores on the scalar-engine ring so
store waits never block load issue. The kernel is memory/DVE balanced:
8 MiB of bf16 loads + 8 MiB of fp32 stores per core overlap ~35 us of
serial VectorEngine scan work.
"""

import sys

import numpy as np

for _p in ("/opt/trn_rl_repo", "/opt/pypackages"):
    if _p not in sys.path:
        sys.path.append(_p)

import concourse.bacc as bacc
import concourse.mybir as mybir
from concourse.bass_utils import run_bass_kernel_spmd
from concourse.tile import TileContext

B, N, D = 4, 4096, 1024
N_CORES = 8
LANES = B * D                        # 4096 independent (b, d) lanes
LANES_PER_CORE = LANES // N_CORES    # 512
P = 128                              # SBUF partitions
TILES_PER_CORE = LANES_PER_CORE // P # 4

TRACE = False       # test harness sets True to capture a neuron-profile trace
# bf16 input storage halves load bytes; the scan's internal state and the
# stored outputs stay fp32, so only input quantization (~2e-3 relative)
# enters — far inside the 2e-2 gate — while cutting ~25% off the runtime.
USE_BF16 = True
# bf16 output stores halve store bytes (8->4 MiB/core): ~4 us faster in
# paired same-session A/B. Adds only terminal output rounding (state stays
# fp32; total rel err 2.57e-3 vs the 2e-2 gate).
BF16_OUT = True
STORE_ENGINE = "scalar"  # dev knob: "scalar" (HWDGE ring) or "gpsimd" (SWDGE)
PACKED = False      # dev knob: host-pack g+x so each chunk loads in one DMA
BACC_OPTS = {}      # dev knob: kwargs for bacc.Bacc()
WARMUP_DMA = False  # dev knob: tiny sacrificial load to absorb DMA spin-up
_result_info = {}   # exec_time_ns / trace path from the last run

# Scan/store segment sizes along N per chunk. Loads are always whole-chunk
# (small DMAs drop well below line rate). The last chunk's final segments
# are smaller to shorten the exposed store tail after the last scan.
_PLAN = [
    [2048, 2048],
    [2048, 2048],
    [2048, 2048],
    [2048, 1024, 512, 512],
]


def _build() -> bacc.Bacc:
    in_dt = mybir.dt.bfloat16 if USE_BF16 else mybir.dt.float32
    nc = bacc.Bacc(**BACC_OPTS)
    if PACKED:
        gx = nc.dram_tensor(
            "gx", [LANES_PER_CORE, 2, N], in_dt, kind="ExternalInput"
        )
    else:
        g = nc.dram_tensor(
            "gates", [LANES_PER_CORE, N], in_dt, kind="ExternalInput"
        )
        x = nc.dram_tensor(
            "inputs", [LANES_PER_CORE, N], in_dt, kind="ExternalInput"
        )
    out_dt = mybir.dt.bfloat16 if BF16_OUT else mybir.dt.float32
    o = nc.dram_tensor("out", [LANES_PER_CORE, N], out_dt, kind="ExternalOutput")
    with TileContext(nc) as tc:
        # bufs=4 holds all four chunks at once (128 KiB/partition), so no
        # load ever waits on a slot release; the sync ring's FIFO still
        # delivers chunk completions in scan order.
        with tc.tile_pool(name="pool", bufs=4) as pool:
            if WARMUP_DMA and not PACKED:
                warm = pool.tile([P, 64], in_dt, tag="warm", name="warm")
                nc.sync.dma_start(out=warm[:, :], in_=g[0:P, 0:64])
            for i, segs in enumerate(_PLAN):
                rows = slice(i * P, (i + 1) * P)
                if PACKED:
                    gxt = pool.tile([P, 2, N], in_dt, tag="gx", name="gxt")
                    gt = gxt[:, 0, :]
                    xt = gxt[:, 1, :]
                else:
                    gt = pool.tile([P, N], in_dt, tag="g", name="gt")
                    xt = pool.tile([P, N], in_dt, tag="x", name="xt")
                # The scan's internal state is fp32 regardless of operand
                # dtype; with a separate fp32 output tile the only bf16 loss
                # is input quantization. fp32 path scans in place (the DVE
                # write trails the read by the pipeline depth).
                if USE_BF16:
                    ot = pool.tile([P, N], out_dt, tag="o", name="ot")
                else:
                    ot = xt
                # Loads on the sync-engine HWDGE ring: FIFO drain makes
                # completions arrive in exactly scan order, so each chunk's
                # scans wait only for their own bytes.
                if PACKED:
                    nc.sync.dma_start(out=gxt[:, :, :], in_=gx[rows, :, :])
                else:
                    nc.sync.dma_start(out=gt[:, :], in_=g[rows, :])
                    nc.sync.dma_start(out=xt[:, :], in_=x[rows, :])
                c0 = 0
                for seg in segs:
                    cols = slice(c0, c0 + seg)
                    init = 0.0 if c0 == 0 else ot[:, c0 - 1 : c0]
                    nc.vector.tensor_tensor_scan(
                        ot[:, cols],
                        gt[:, cols],
                        xt[:, cols],
                        init,
                        mybir.AluOpType.mult,
                        mybir.AluOpType.add,
                    )
                    # Stores ride a ring separate from the loads so their
                    # waits never stall load issue on the sync ring.
                    store_eng = (
                        nc.gpsimd if STORE_ENGINE == "gpsimd" else nc.scalar
                    )
                    store_eng.dma_start(out=o[rows, cols], in_=ot[:, cols])
                    c0 += seg
    nc.compile()
    return nc


def kernel(gates: np.ndarray, inputs: np.ndarray) -> np.ndarray:
    gates = np.asarray(gates, dtype=np.float32)
    inputs = np.asarray(inputs, dtype=np.float32)

    # Host-side shard: (B, N, D) -> lane-major (B*D, N); row b*D + d is the
    # contiguous time series of lane (b, d).
    gt = np.ascontiguousarray(gates.transpose(0, 2, 1)).reshape(LANES, N)
    xt = np.ascontiguousarray(inputs.transpose(0, 2, 1)).reshape(LANES, N)
    if USE_BF16:
        import ml_dtypes

        gt = gt.astype(ml_dtypes.bfloat16)
        xt = xt.astype(ml_dtypes.bfloat16)

    in_maps = []
    if PACKED:
        gx = np.empty((LANES, 2, N), dtype=gt.dtype)
        gx[:, 0, :] = gt
        gx[:, 1, :] = xt
        for c in range(N_CORES):
            rows = slice(c * LANES_PER_CORE, (c + 1) * LANES_PER_CORE)
            in_maps.append({"gx": gx[rows]})
    else:
        for c in range(N_CORES):
            rows = slice(c * LANES_PER_CORE, (c + 1) * LANES_PER_CORE)
            in_maps.append({"gates": gt[rows], "inputs": xt[rows]})

    nc = _build()
    res = run_bass_kernel_spmd(
        nc, in_maps, core_ids=list(range(N_CORES)), trace=TRACE
    )
    _result_info["exec_time_ns"] = res.exec_time_ns
    _result_info["mean_exec_time_ns"] = res.mean_exec_time_ns
    _result_info["profile_json"] = res.profile_json
    _result_info["trace"] = (
        res.instructions_and_trace[1] if res.instructions_and_trace else None
    )

    out_t = np.concatenate([r["out"] for r in res.results], axis=0)  # (LANES, N)
    out_t = out_t.astype(np.float32, copy=False)
    return np.ascontiguousarray(out_t.reshape(B, D, N).transpose(0, 2, 1))



# revision 2
# speedup vs baseline: 1.0475x; 1.0475x over previous
"""AssocScan Trainium2 kernel: out[:, t] = gates[:, t] * out[:, t-1] + inputs[:, t].

Strategy: the recurrence is independent per (b, d) lane (B*D = 4096 lanes,
N = 4096 steps). The DVE `tensor_tensor_scan` instruction computes exactly
this recurrence along the free dimension, one lane per partition, at a
measured ~2.09 ns/column — that serial rate is the kernel's hard floor
(no other engine supports the scan opcode on NeuronCore v3).

Layout: lanes are split across the 8 cores (512 lanes each). On the host,
each core's 512 lanes are packed 4-per-partition, concatenated along the
free dim into one [128, 16384] stream. Because g[:, 0] of every lane
never affects the result (it multiplies the zero initial state), the host
zeroes it; the scan state then self-resets at each lane boundary, so the
whole stream can be scanned with a handful of long chained
tensor_tensor_scan instructions (long scans amortize the per-instruction
overhead; chaining passes the carry through the previous segment's last
output column).

Pipeline: column segments sized small at the head (scan starts as soon as
the first ~512 columns land) and tapered at the tail (short final store).
Gate loads ride the sync-engine HWDGE ring, input loads the gpsimd SWDGE
ring (two rings ≈ 400 B/ns aggregate, ahead of the scan's ~245 B/ns
consumption), stores the scalar-engine ring.
"""

import sys

import numpy as np

for _p in ("/opt/trn_rl_repo", "/opt/pypackages"):
    if _p not in sys.path:
        sys.path.append(_p)

import concourse.bacc as bacc
import concourse.mybir as mybir
from concourse.bass_utils import run_bass_kernel_spmd
from concourse.tile import TileContext

B, N, D = 4, 4096, 1024
N_CORES = 8
LANES = B * D                        # 4096 independent (b, d) lanes
LANES_PER_CORE = LANES // N_CORES    # 512
P = 128                              # SBUF partitions
LPP = LANES_PER_CORE // P            # 4 lanes per partition
NC = LPP * N                         # 16384 columns per partition

TRACE = False       # test harness sets True to capture a neuron-profile trace
USE_BF16 = True     # bf16 inputs: quantization ~2e-3 rel, halves load bytes
BF16_OUT = True     # bf16 output stores: halves store bytes
_result_info = {}   # exec_time_ns / trace path from the last run

# Column segment sizes (sum = NC). Small head segments let the scan start
# as soon as the first columns land; the short tail segment keeps the
# final store off the critical path.
_SEGS = [512, 1024, 2048, 4096, 4096, 4096, 512]
assert sum(_SEGS) == NC


def _build() -> bacc.Bacc:
    in_dt = mybir.dt.bfloat16 if USE_BF16 else mybir.dt.float32
    out_dt = mybir.dt.bfloat16 if BF16_OUT else mybir.dt.float32
    nc = bacc.Bacc()
    g = nc.dram_tensor("gates", [P, NC], in_dt, kind="ExternalInput")
    x = nc.dram_tensor("inputs", [P, NC], in_dt, kind="ExternalInput")
    o = nc.dram_tensor("out", [P, NC], out_dt, kind="ExternalOutput")
    M = mybir.AluOpType.mult
    A = mybir.AluOpType.add
    with TileContext(nc) as tc:
        with tc.tile_pool(name="pool", bufs=1) as pool:
            gts, xts, ots = [], [], []
            for k, seg in enumerate(_SEGS):
                gts.append(pool.tile([P, seg], in_dt, name=f"g{k}"))
                xts.append(pool.tile([P, seg], in_dt, name=f"x{k}"))
                ots.append(pool.tile([P, seg], out_dt, name=f"o{k}"))
            # Issue every load up front: g on the sync HWDGE ring, x on the
            # gpsimd SWDGE ring. All tiles coexist in SBUF (96 KiB/partition),
            # so nothing waits on a buffer release.
            c0 = 0
            for k, seg in enumerate(_SEGS):
                nc.sync.dma_start(out=gts[k][:, :], in_=g[:, c0 : c0 + seg])
                nc.gpsimd.dma_start(out=xts[k][:, :], in_=x[:, c0 : c0 + seg])
                c0 += seg
            # Chained scans; carry crosses segment boundaries through the
            # previous segment's last output column (bf16 rounding there is
            # far inside the error budget). Lane resets happen wherever the
            # host zeroed the gate.
            c0 = 0
            for k, seg in enumerate(_SEGS):
                init = 0.0 if k == 0 else ots[k - 1][:, -1:]
                nc.vector.tensor_tensor_scan(
                    ots[k][:, :], gts[k][:, :], xts[k][:, :], init, M, A
                )
                nc.scalar.dma_start(out=o[:, c0 : c0 + seg], in_=ots[k][:, :])
                c0 += seg
    nc.compile()
    return nc


def kernel(gates: np.ndarray, inputs: np.ndarray) -> np.ndarray:
    gates = np.asarray(gates, dtype=np.float32)
    inputs = np.asarray(inputs, dtype=np.float32)

    # Host-side shard: (B, N, D) -> lane-major (B*D, N); row b*D + d is the
    # contiguous time series of lane (b, d). The first gate of every lane
    # multiplies the zero initial state, so it is dead — zero it to make
    # the scan state reset at lane boundaries after concatenation.
    gt = np.ascontiguousarray(gates.transpose(0, 2, 1)).reshape(LANES, N)
    xt = np.ascontiguousarray(inputs.transpose(0, 2, 1)).reshape(LANES, N)
    gt[:, 0] = 0.0
    if USE_BF16:
        import ml_dtypes

        gt = gt.astype(ml_dtypes.bfloat16)
        xt = xt.astype(ml_dtypes.bfloat16)

    # Per core: [512, N] -> [LPP, P, N] -> [P, LPP, N] -> [P, NC]: partition
    # p holds lanes {base + p, base + P + p, ...} concatenated in time.
    in_maps = []
    for c in range(N_CORES):
        rows = slice(c * LANES_PER_CORE, (c + 1) * LANES_PER_CORE)
        gc = (
            gt[rows].reshape(LPP, P, N).transpose(1, 0, 2).reshape(P, NC)
        )
        xc = (
            xt[rows].reshape(LPP, P, N).transpose(1, 0, 2).reshape(P, NC)
        )
        in_maps.append(
            {"gates": np.ascontiguousarray(gc), "inputs": np.ascontiguousarray(xc)}
        )

    nc = _build()
    res = run_bass_kernel_spmd(
        nc, in_maps, core_ids=list(range(N_CORES)), trace=TRACE
    )
    _result_info["exec_time_ns"] = res.exec_time_ns
    _result_info["mean_exec_time_ns"] = res.mean_exec_time_ns
    _result_info["profile_json"] = res.profile_json
    _result_info["trace"] = (
        res.instructions_and_trace[1] if res.instructions_and_trace else None
    )

    # Undo the per-core packing: [P, NC] -> [P, LPP, N] -> [LPP, P, N] ->
    # [512, N], then stack cores back to (LANES, N).
    parts = []
    for c in range(N_CORES):
        oc = res.results[c]["out"].astype(np.float32, copy=False)
        parts.append(
            oc.reshape(P, LPP, N).transpose(1, 0, 2).reshape(LANES_PER_CORE, N)
        )
    out_t = np.concatenate(parts, axis=0)  # (LANES, N)
    return np.ascontiguousarray(out_t.reshape(B, D, N).transpose(0, 2, 1))


# revision 5
# speedup vs baseline: 1.0914x; 1.0420x over previous
"""AssocScan Trainium2 kernel: out[:, t] = gates[:, t] * out[:, t-1] + inputs[:, t].

Strategy: the recurrence is independent per (b, d) lane (B*D = 4096 lanes,
N = 4096 steps). The DVE `tensor_tensor_scan` instruction computes exactly
this recurrence along the free dimension, one lane per partition, at a
measured ~2.09 ns/column — that serial rate is the kernel's hard floor
(no other engine supports the scan opcode on NeuronCore v3).

Layout: lanes are split across the 8 cores (512 lanes each). On the host,
each core's 512 lanes are packed 4-per-partition, concatenated along the
free dim into one [128, 16384] stream. Because g[:, 0] of every lane
never affects the result (it multiplies the zero initial state), the host
zeroes it; the scan state then self-resets at each lane boundary, so the
whole stream can be scanned with a handful of long chained
tensor_tensor_scan instructions (long scans amortize the per-instruction
overhead; chaining passes the carry through the previous segment's last
output column).

Pipeline: column segments sized small at the head (scan starts as soon as
the first ~512 columns land) and tapered at the tail (short final store).
Gate loads ride the sync-engine HWDGE ring, input loads the gpsimd SWDGE
ring (two rings ≈ 400 B/ns aggregate, ahead of the scan's ~245 B/ns
consumption), stores the scalar-engine ring.
"""

import sys

import numpy as np

for _p in ("/opt/trn_rl_repo", "/opt/pypackages"):
    if _p not in sys.path:
        sys.path.append(_p)

import concourse.bacc as bacc
import concourse.mybir as mybir
from concourse.bass_utils import run_bass_kernel_spmd
from concourse.tile import TileContext

B, N, D = 4, 4096, 1024
N_CORES = 8
LANES = B * D                        # 4096 independent (b, d) lanes
LANES_PER_CORE = LANES // N_CORES    # 512
P = 128                              # SBUF partitions
LPP = LANES_PER_CORE // P            # 4 lanes per partition
NC = LPP * N                         # 16384 columns per partition

TRACE = False       # test harness sets True to capture a neuron-profile trace
USE_BF16 = True     # bf16 inputs: quantization ~2e-3 rel, halves load bytes
BF16_OUT = True     # bf16 output stores: halves store bytes
_result_info = {}   # exec_time_ns / trace path from the last run

# Column segment sizes (sum = NC). Small head segments let the scan start
# as soon as the first columns land; the short tail segment keeps the
# final store off the critical path.
_SEGS = [512, 1024, 2048, 4096, 4096, 4096, 512]
assert sum(_SEGS) == NC


def _build() -> bacc.Bacc:
    in_dt = mybir.dt.bfloat16 if USE_BF16 else mybir.dt.float32
    out_dt = mybir.dt.bfloat16 if BF16_OUT else mybir.dt.float32
    nc = bacc.Bacc()
    # One contiguous DRAM tensor per segment: every DMA source/dest is a
    # single dense block, which keeps the queues at full descriptor
    # efficiency (column-slicing one big [P, NC] tensor dropped the load
    # rings to ~140 B/ns; dense blocks run ~290 B/ns).
    gs = [
        nc.dram_tensor(f"g{k}", [P, seg], in_dt, kind="ExternalInput")
        for k, seg in enumerate(_SEGS)
    ]
    xs = [
        nc.dram_tensor(f"x{k}", [P, seg], in_dt, kind="ExternalInput")
        for k, seg in enumerate(_SEGS)
    ]
    os_ = [
        nc.dram_tensor(f"o{k}", [P, seg], out_dt, kind="ExternalOutput")
        for k, seg in enumerate(_SEGS)
    ]
    M = mybir.AluOpType.mult
    A = mybir.AluOpType.add
    with TileContext(nc) as tc:
        with tc.tile_pool(name="pool", bufs=1) as pool:
            gts, xts, ots = [], [], []
            for k, seg in enumerate(_SEGS):
                gts.append(pool.tile([P, seg], in_dt, name=f"gt{k}"))
                xts.append(pool.tile([P, seg], in_dt, name=f"xt{k}"))
                ots.append(pool.tile([P, seg], out_dt, name=f"ot{k}"))
            # Issue every load up front: g on the sync HWDGE ring, x on the
            # scalar HWDGE ring. All tiles coexist in SBUF (96 KiB/partition),
            # so nothing waits on a buffer release.
            for k, seg in enumerate(_SEGS):
                nc.sync.dma_start(out=gts[k][:, :], in_=gs[k][:, :])
                nc.scalar.dma_start(out=xts[k][:, :], in_=xs[k][:, :])
            # Chained scans; carry crosses segment boundaries through the
            # previous segment's last output column (bf16 rounding there is
            # far inside the error budget). Lane resets happen wherever the
            # host zeroed the gate. Stores ride the gpsimd SWDGE ring so
            # they never contend with load dispatch.
            for k, seg in enumerate(_SEGS):
                init = 0.0 if k == 0 else ots[k - 1][:, -1:]
                nc.vector.tensor_tensor_scan(
                    ots[k][:, :], gts[k][:, :], xts[k][:, :], init, M, A
                )
                nc.gpsimd.dma_start(out=os_[k][:, :], in_=ots[k][:, :])
    nc.compile()
    return nc


def kernel(gates: np.ndarray, inputs: np.ndarray) -> np.ndarray:
    gates = np.asarray(gates, dtype=np.float32)
    inputs = np.asarray(inputs, dtype=np.float32)

    # Host-side shard: (B, N, D) -> lane-major (B*D, N); row b*D + d is the
    # contiguous time series of lane (b, d). The first gate of every lane
    # multiplies the zero initial state, so it is dead — zero it to make
    # the scan state reset at lane boundaries after concatenation.
    gt = np.ascontiguousarray(gates.transpose(0, 2, 1)).reshape(LANES, N)
    xt = np.ascontiguousarray(inputs.transpose(0, 2, 1)).reshape(LANES, N)
    gt[:, 0] = 0.0
    if USE_BF16:
        import ml_dtypes

        gt = gt.astype(ml_dtypes.bfloat16)
        xt = xt.astype(ml_dtypes.bfloat16)

    # Per core: [512, N] -> [LPP, P, N] -> [P, LPP, N] -> [P, NC]: partition
    # p holds lanes {base + p, base + P + p, ...} concatenated in time.
    # Each column segment ships as its own contiguous array.
    bounds = np.cumsum([0] + _SEGS)
    in_maps = []
    for c in range(N_CORES):
        rows = slice(c * LANES_PER_CORE, (c + 1) * LANES_PER_CORE)
        gc = gt[rows].reshape(LPP, P, N).transpose(1, 0, 2).reshape(P, NC)
        xc = xt[rows].reshape(LPP, P, N).transpose(1, 0, 2).reshape(P, NC)
        m = {}
        for k in range(len(_SEGS)):
            sl = slice(bounds[k], bounds[k + 1])
            m[f"g{k}"] = np.ascontiguousarray(gc[:, sl])
            m[f"x{k}"] = np.ascontiguousarray(xc[:, sl])
        in_maps.append(m)

    nc = _build()
    res = run_bass_kernel_spmd(
        nc, in_maps, core_ids=list(range(N_CORES)), trace=TRACE
    )
    _result_info["exec_time_ns"] = res.exec_time_ns
    _result_info["mean_exec_time_ns"] = res.mean_exec_time_ns
    _result_info["profile_json"] = res.profile_json
    _result_info["trace"] = (
        res.instructions_and_trace[1] if res.instructions_and_trace else None
    )

    # Undo the per-core packing: concat segments -> [P, NC] -> [P, LPP, N]
    # -> [LPP, P, N] -> [512, N], then stack cores back to (LANES, N).
    parts = []
    for c in range(N_CORES):
        oc = np.concatenate(
            [
                res.results[c][f"o{k}"].astype(np.float32, copy=False)
                for k in range(len(_SEGS))
            ],
            axis=1,
        )
        parts.append(
            oc.reshape(P, LPP, N).transpose(1, 0, 2).reshape(LANES_PER_CORE, N)
        )
    out_t = np.concatenate(parts, axis=0)  # (LANES, N)
    return np.ascontiguousarray(out_t.reshape(B, D, N).transpose(0, 2, 1))


# revision 8
# speedup vs baseline: 1.1446x; 1.0488x over previous
"""AssocScan Trainium2 kernel: out[:, t] = gates[:, t] * out[:, t-1] + inputs[:, t].

Strategy: the recurrence is independent per (b, d) lane (B*D = 4096 lanes,
N = 4096 steps). The DVE `tensor_tensor_scan` instruction computes exactly
this recurrence along the free dimension, one lane per partition, at a
measured ~2.09 ns/column — that serial rate is the kernel's hard floor
(no other engine supports the scan opcode on NeuronCore v3).

Layout: lanes are split across the 8 cores (512 lanes each). On the host,
each core's 512 lanes are packed 4-per-partition, concatenated along the
free dim into one [128, 16384] stream. Because g[:, 0] of every lane
never affects the result (it multiplies the zero initial state), the host
zeroes it; the scan state then self-resets at each lane boundary, so the
whole stream can be scanned with a handful of long chained
tensor_tensor_scan instructions (long scans amortize the per-instruction
overhead; chaining passes the carry through the previous segment's last
output column).

Pipeline: column segments sized small at the head (scan starts as soon as
the first ~512 columns land) and tapered at the tail (short final store).
Gate loads ride the sync-engine HWDGE ring, input loads the gpsimd SWDGE
ring (two rings ≈ 400 B/ns aggregate, ahead of the scan's ~245 B/ns
consumption), stores the scalar-engine ring.
"""

import sys

import numpy as np

for _p in ("/opt/trn_rl_repo", "/opt/pypackages"):
    if _p not in sys.path:
        sys.path.append(_p)

import concourse.bacc as bacc
import concourse.mybir as mybir
from concourse.bass_utils import run_bass_kernel_spmd
from concourse.tile import TileContext

B, N, D = 4, 4096, 1024
N_CORES = 8
LANES = B * D                        # 4096 independent (b, d) lanes
LANES_PER_CORE = LANES // N_CORES    # 512
P = 128                              # SBUF partitions
LPP = LANES_PER_CORE // P            # 4 lanes per partition
NC = LPP * N                         # 16384 columns per partition

TRACE = False       # test harness sets True to capture a neuron-profile trace
USE_BF16 = True     # bf16 inputs: quantization ~2e-3 rel, halves load bytes
BF16_OUT = True     # bf16 output stores: halves store bytes
_result_info = {}   # exec_time_ns / trace path from the last run

# Column segment sizes (sum = NC). Small head segments let the scan start
# as soon as the first columns land; the short tail segment keeps the
# final store off the critical path.
import os as _os

_SEGS = [int(s) for s in _os.environ.get(
    "SEGS", "512,1024,2048,4096,4096,4096,512"
).split(",")]
assert sum(_SEGS) == NC
# LOAD_SPLIT=1: g on sync ring, x on scalar ring. 0: both on sync ring
# (g and x interleaved per segment, FIFO completion in scan order).
_LOAD_SPLIT = _os.environ.get("LOAD_SPLIT", "1") == "1"
_STORE_ENG = _os.environ.get("STORE_ENG", "gpsimd")


def _build() -> bacc.Bacc:
    in_dt = mybir.dt.bfloat16 if USE_BF16 else mybir.dt.float32
    out_dt = mybir.dt.bfloat16 if BF16_OUT else mybir.dt.float32
    nc = bacc.Bacc()
    # One contiguous DRAM tensor per segment: every DMA source/dest is a
    # single dense block, which keeps the queues at full descriptor
    # efficiency (column-slicing one big [P, NC] tensor dropped the load
    # rings to ~140 B/ns; dense blocks run ~290 B/ns).
    gs = [
        nc.dram_tensor(f"g{k}", [P, seg], in_dt, kind="ExternalInput")
        for k, seg in enumerate(_SEGS)
    ]
    xs = [
        nc.dram_tensor(f"x{k}", [P, seg], in_dt, kind="ExternalInput")
        for k, seg in enumerate(_SEGS)
    ]
    os_ = [
        nc.dram_tensor(f"o{k}", [P, seg], out_dt, kind="ExternalOutput")
        for k, seg in enumerate(_SEGS)
    ]
    M = mybir.AluOpType.mult
    A = mybir.AluOpType.add
    with TileContext(nc) as tc:
        with tc.tile_pool(name="pool", bufs=1) as pool:
            gts, xts, ots = [], [], []
            for k, seg in enumerate(_SEGS):
                gts.append(pool.tile([P, seg], in_dt, name=f"gt{k}"))
                xts.append(pool.tile([P, seg], in_dt, name=f"xt{k}"))
                ots.append(pool.tile([P, seg], out_dt, name=f"ot{k}"))
            # Issue every load up front: g on the sync HWDGE ring, x on the
            # scalar HWDGE ring. All tiles coexist in SBUF (96 KiB/partition),
            # so nothing waits on a buffer release.
            x_eng = nc.scalar if _LOAD_SPLIT else nc.sync
            for k, seg in enumerate(_SEGS):
                nc.sync.dma_start(out=gts[k][:, :], in_=gs[k][:, :])
                x_eng.dma_start(out=xts[k][:, :], in_=xs[k][:, :])
            # Chained scans; carry crosses segment boundaries through the
            # previous segment's last output column (bf16 rounding there is
            # far inside the error budget). Lane resets happen wherever the
            # host zeroed the gate. Stores ride the gpsimd SWDGE ring so
            # they never contend with load dispatch.
            store_eng = {
                "gpsimd": nc.gpsimd,
                "scalar": nc.scalar,
                "sync": nc.sync,
            }[_STORE_ENG]
            for k, seg in enumerate(_SEGS):
                init = 0.0 if k == 0 else ots[k - 1][:, -1:]
                nc.vector.tensor_tensor_scan(
                    ots[k][:, :], gts[k][:, :], xts[k][:, :], init, M, A
                )
                store_eng.dma_start(out=os_[k][:, :], in_=ots[k][:, :])
    nc.compile()
    return nc


def kernel(gates: np.ndarray, inputs: np.ndarray) -> np.ndarray:
    gates = np.asarray(gates, dtype=np.float32)
    inputs = np.asarray(inputs, dtype=np.float32)

    # Host-side shard: (B, N, D) -> lane-major (B*D, N); row b*D + d is the
    # contiguous time series of lane (b, d). The first gate of every lane
    # multiplies the zero initial state, so it is dead — zero it to make
    # the scan state reset at lane boundaries after concatenation.
    gt = np.ascontiguousarray(gates.transpose(0, 2, 1)).reshape(LANES, N)
    xt = np.ascontiguousarray(inputs.transpose(0, 2, 1)).reshape(LANES, N)
    gt[:, 0] = 0.0
    if USE_BF16:
        import ml_dtypes

        gt = gt.astype(ml_dtypes.bfloat16)
        xt = xt.astype(ml_dtypes.bfloat16)

    # Per core: [512, N] -> [LPP, P, N] -> [P, LPP, N] -> [P, NC]: partition
    # p holds lanes {base + p, base + P + p, ...} concatenated in time.
    # Each column segment ships as its own contiguous array.
    bounds = np.cumsum([0] + _SEGS)
    in_maps = []
    for c in range(N_CORES):
        rows = slice(c * LANES_PER_CORE, (c + 1) * LANES_PER_CORE)
        gc = gt[rows].reshape(LPP, P, N).transpose(1, 0, 2).reshape(P, NC)
        xc = xt[rows].reshape(LPP, P, N).transpose(1, 0, 2).reshape(P, NC)
        m = {}
        for k in range(len(_SEGS)):
            sl = slice(bounds[k], bounds[k + 1])
            m[f"g{k}"] = np.ascontiguousarray(gc[:, sl])
            m[f"x{k}"] = np.ascontiguousarray(xc[:, sl])
        in_maps.append(m)

    nc = _build()
    res = run_bass_kernel_spmd(
        nc, in_maps, core_ids=list(range(N_CORES)), trace=TRACE
    )
    _result_info["exec_time_ns"] = res.exec_time_ns
    _result_info["mean_exec_time_ns"] = res.mean_exec_time_ns
    _result_info["profile_json"] = res.profile_json
    _result_info["trace"] = (
        res.instructions_and_trace[1] if res.instructions_and_trace else None
    )

    # Undo the per-core packing: concat segments -> [P, NC] -> [P, LPP, N]
    # -> [LPP, P, N] -> [512, N], then stack cores back to (LANES, N).
    parts = []
    for c in range(N_CORES):
        oc = np.concatenate(
            [
                res.results[c][f"o{k}"].astype(np.float32, copy=False)
                for k in range(len(_SEGS))
            ],
            axis=1,
        )
        parts.append(
            oc.reshape(P, LPP, N).transpose(1, 0, 2).reshape(LANES_PER_CORE, N)
        )
    out_t = np.concatenate(parts, axis=0)  # (LANES, N)
    return np.ascontiguousarray(out_t.reshape(B, D, N).transpose(0, 2, 1))
